# revision 20
# baseline (speedup 1.0000x reference)
"""Multi-head attention (unfused, returns attn weights) on 8 TRN2 NeuronCores.

Problem: Q,K,V [16, 2048, 128] fp32 ->
  out          [16, 2048, 128] fp32  (softmax(QK^T/sqrt(d)) @ V)
  attn_weights [16, 2048, 2048] fp32

Sharding: batch dim across 8 cores, 2 batches/core, no cross-core comm.

Per-core design ("dual matmul", no big transposes):
  - Q,K cast to bf16 (SWDGE cast-DMA) and PE-transposed once to [d, s]
    layout (QT, KT).
  - Per q-tile i (128 rows):
      S_i  = QT_i^T @ KT    [128q, 2048k] fp32 in PSUM   (4 matmuls N=512)
      exp1: E_i = exp(S_i/sqrt(d)) -> fp32 SBUF, accum_out = rowsums
      P_i  = E_i * (1/rowsum)      -> fp32 SBUF (DVE tensor_scalar, fp32 fast path)
      DMA P_i -> attn (1 MB contiguous store)
      ST_i = KT_i^T @ QT   [128k, 2048q] in two 1024 halves (recompute beats
             transposing E); exp2 -> ET_i bf16 SBUF (k-major layout for AV)
  - AV (deferred one batch so ET is complete): per 512-wide q-chunk,
      outT = sum_k V_k lhsT-matmul ET_k -> [128d, 512q] PSUM, PE-transpose
      back to [q, d], scale rows by 1/rowsum.
  - PSUM: pS 4 banks (S, bufs=1) + pST 4 banks (2-slot ring shared by ST
    halves, AV accum, and all transpose targets) = 8 banks.
"""

import sys

if "/opt/trn_rl_repo" not in sys.path:
    sys.path.insert(0, "/opt/trn_rl_repo")

from contextlib import ExitStack

import numpy as np

from concourse import bacc, bass, mybir
from concourse.bass import ts
from concourse.bass_utils import run_bass_kernel_spmd
from concourse.masks import make_identity
from concourse.tile import TileContext

F32 = mybir.dt.float32
BF16 = mybir.dt.bfloat16
EXP = mybir.ActivationFunctionType.Exp
MULT = mybir.AluOpType.mult

N_CORES = 8
B_FULL = 16
BPC = B_FULL // N_CORES  # batches per core
S = 2048
D = 128
NT = S // 128  # 16 q/k tiles per batch
SCALE = float(1.0 / np.sqrt(D))


def build_graph():
    nc = bacc.Bacc("TRN2", target_bir_lowering=False, debug=False, num_devices=N_CORES)
    Q = nc.dram_tensor("Q", [BPC, S, D], F32, kind="ExternalInput").ap()
    K = nc.dram_tensor("K", [BPC, S, D], F32, kind="ExternalInput").ap()
    V = nc.dram_tensor("V", [BPC, S, D], F32, kind="ExternalInput").ap()
    out = nc.dram_tensor("out", [BPC, S, D], F32, kind="ExternalOutput").ap()
    attn = nc.dram_tensor("attn", [BPC, S, S], F32, kind="ExternalOutput").ap()

    with TileContext(nc) as tc, ExitStack() as ctx:
        const_pool = ctx.enter_context(tc.tile_pool(name="const", bufs=1))
        ident_bf = const_pool.tile([128, 128], BF16)
        ident_f32 = const_pool.tile([128, 128], F32)
        make_identity(nc, ident_bf[:])
        make_identity(nc, ident_f32[:])

        # bf16 copies of inputs, [128, NT, 128]; partition = s % 128
        p_qkv = ctx.enter_context(tc.tile_pool(name="qkv16", bufs=2))
        p_qt = ctx.enter_context(tc.tile_pool(name="qt", bufs=2))
        p_kt = ctx.enter_context(tc.tile_pool(name="kt", bufs=2))
        p_v = ctx.enter_context(tc.tile_pool(name="v", bufs=2))
        # PSUM: pS 4 banks + pST ring 2 slots x 2 banks = 8 banks
        p_s = ctx.enter_context(tc.tile_pool(name="ps", bufs=1, space="PSUM"))
        p_st = ctx.enter_context(tc.tile_pool(name="pst", bufs=2, space="PSUM"))
        # SBUF working pools
        p_e = ctx.enter_context(tc.tile_pool(name="pe_", bufs=2))
        p_p = ctx.enter_context(tc.tile_pool(name="pp", bufs=3))
        p_et = ctx.enter_context(tc.tile_pool(name="pet", bufs=NT + 13))
        p_rs = ctx.enter_context(tc.tile_pool(name="prs", bufs=3))
        p_rr = ctx.enter_context(tc.tile_pool(name="prr", bufs=2 * NT + 4))
        p_avsb = ctx.enter_context(tc.tile_pool(name="pavsb", bufs=2))
        p_osb = ctx.enter_context(tc.tile_pool(name="posb", bufs=2))

        state = {}

        def st_tile(shape, dtype):
            # all PSUM users besides S share the 2-slot "st" ring
            return p_st.tile(shape, dtype, tag="st", name="stslot")

        def prep_loads(b, fast=False):
            """Load Q/K/V[b] as bf16 [128, NT, D]. fast=True (first batch):
            fp32 halves across BOTH parallel HWDGE rings (sync + scalar) +
            DVE casts — SWDGE desc-gen serializes ~1.3us per DMA on GpSimd,
            which would push Q's arrival to ~19us. V rides SWDGE cast-DMA."""
            qb = p_qkv.tile([128, NT, D], BF16, tag="qkv")
            kb = p_qkv.tile([128, NT, D], BF16, tag="qkv")
            vb = p_v.tile([128, NT, D], BF16)
            h = NT // 2
            rk = K[b].rearrange("(t p) d -> p t d", p=128)
            rq = Q[b].rearrange("(t p) d -> p t d", p=128)
            if fast:
                for r, dst in ((rk, kb), (rq, qb)):
                    for hi, eng in ((0, nc.sync), (1, nc.scalar)):
                        raw = p_p.tile([128, h, D], F32, tag="p_sb", name="raw")
                        sl = slice(hi * h, (hi + 1) * h)
                        eng.dma_start(out=raw[:], in_=r[:, sl, :])
                        nc.vector.tensor_copy(dst[:, sl, :], raw[:])
            else:
                nc.gpsimd.dma_start(out=kb[:, :h, :], in_=rk[:, :h, :])
                nc.gpsimd.dma_start(out=kb[:, h:, :], in_=rk[:, h:, :])
                nc.gpsimd.dma_start(out=qb[:, :h, :], in_=rq[:, :h, :])
                nc.gpsimd.dma_start(out=qb[:, h:, :], in_=rq[:, h:, :])
            nc.gpsimd.dma_start(out=vb[:], in_=V[b].rearrange("(t p) d -> p t d", p=128))
            return qb, kb, vb

        def prep_group(src, dst, g):
            """PE-transpose one group of 4 [128,128] tiles into dst[:, g*512:]."""
            tr = st_tile([128, 512], BF16)
            for m in range(4):
                t = g * 4 + m
                nc.tensor.transpose(tr[:, ts(m, 128)], src[:, t, :], ident_bf[:])
            nc.vector.tensor_copy(dst[:, ts(g, 512)], tr[:])

        def prep_build(qb, kb):
            """PE-transpose bf16 Q/K tiles into [d, s] layout."""
            qt = p_qt.tile([128, S], BF16)
            kt = p_kt.tile([128, S], BF16)
            for src, dst in ((kb, kt), (qb, qt)):  # K first: S_0 needs all of K
                for g in range(NT // 4):
                    prep_group(src, dst, g)
            return qt, kt

        def emit_st(b, i, qt, kt):
            """ST_i + exp2 -> ET_i (two 1024 halves through the st ring)."""
            et = p_et.tile([128, S], BF16)
            state[b]["et"][i] = et
            for h in range(2):
                st_ps = st_tile([128, 1024], F32)
                for c in range(2):
                    nc.tensor.matmul(
                        st_ps[:, ts(c, 512)],
                        lhsT=kt[:, ts(i, 128)],
                        rhs=qt[:, ts(2 * h + c, 512)],
                        start=True,
                        stop=True,
                    )
                nc.scalar.activation(et[:, ts(h, 1024)], st_ps[:], EXP, scale=SCALE)

        def emit_s(b, i, qt, kt):
            """S_i -> exp1 (+rowsum) -> normalize -> P_i store."""
            s_ps = p_s.tile([128, S], F32)
            for c in range(4):
                nc.tensor.matmul(
                    s_ps[:, ts(c, 512)],
                    lhsT=qt[:, ts(i, 128)],
                    rhs=kt[:, ts(c, 512)],
                    start=True,
                    stop=True,
                )
            e_sb = p_e.tile([128, S], F32)
            rs = p_rs.tile([128, 1], F32)
            nc.scalar.activation(e_sb[:], s_ps[:], EXP, scale=SCALE, accum_out=rs[:])
            rr = p_rr.tile([128, 1], F32)
            nc.vector.reciprocal(rr[:], rs[:])
            state[b]["rr"][i] = rr
            p_sb = p_p.tile([128, S], F32)
            nc.vector.tensor_scalar(p_sb[:], e_sb[:], rr[:], None, op0=MULT)
            nc.sync.dma_start(out=attn[b, ts(i, 128), :], in_=p_sb[:])

        def emit_av_start(b, q0, n):
            """AV accum for batch b, q-tiles [q0, q0+n): outT [128d, n*128q]."""
            st = state[b]
            w = n * 128
            av = st_tile([128, w], F32)
            for kt_i in range(NT):
                nc.tensor.matmul(
                    av[:],
                    lhsT=st["v"][:, kt_i, :],
                    rhs=st["et"][kt_i][:, q0 * 128 : q0 * 128 + w],
                    start=(kt_i == 0),
                    stop=(kt_i == NT - 1),
                )
            avsb = p_avsb.tile([128, w], F32, tag="avsb", name="avsb")
            nc.vector.tensor_copy(avsb[:], av[:])
            return (b, q0, n, avsb)

        def emit_av_finish(pend):
            """Transpose outT back to [q, d], scale by 1/rowsum, store.
            The transpose target borrows the S pool's slot (its idle window
            between exp1 reads absorbs the ~2us hold almost for free)."""
            b, q0, n, avsb = pend
            st = state[b]
            otr = st_tile([128, n * 128], F32)
            for m in range(n):
                nc.tensor.transpose(otr[:, ts(m, 128)], avsb[:, ts(m, 128)], ident_f32[:])
            osb = p_osb.tile([128, n, D], F32, tag="osb", name="osb")
            for m in range(n):
                nc.vector.tensor_scalar(
                    osb[:, m, :], otr[:, ts(m, 128)], st["rr"][q0 + m][:], None, op0=MULT
                )
            nc.sync.dma_start(
                out=out[b, q0 * 128 : (q0 + n) * 128, :].rearrange(
                    "(m p) d -> p m d", p=128
                ),
                in_=osb[:],
            )

        # schedules: per batch, which STs / AV starts run on which iteration.
        # AV is split: accumulation+evac at iter I, transpose+store at I+1,
        # so the PSUM ring slot isn't held across an entire epoch.
        last = BPC - 1
        st_sched = {b: {i: [i] for i in range(NT)} for b in range(BPC)}
        av_sched = {b: {} for b in range(BPC)}  # iter -> (batch, q0, ntiles)
        if BPC > 1:
            # last batch: front-load STs (2/iter for i<8) so its own AV
            # chunks can start before the loop ends
            st_sched[last] = {
                i: ([2 * i, 2 * i + 1] if i < 8 else []) for i in range(NT)
            }
            # AV(prev) early (frees prev ET slots before the front-loaded ET
            # tiles peak) but off the congested batch boundary (iter 0 runs
            # prep_build); AV(last) late (needs rr + full ET)
            av_sched[last] = {
                1: (last - 1, 0, 4), 2: (last - 1, 4, 4), 3: (last - 1, 8, 4),
                4: (last - 1, 12, 4), 8: (last, 0, 4), 10: (last, 4, 4),
                12: (last, 8, 4), 14: (last, 12, 2),
            }

        loads = {0: prep_loads(0, fast=True)}
        built = {}
        pending = []
        for b in range(BPC):
            if b not in built:
                qb, kb, vb = loads[b]
                built[b] = prep_build(qb, kb)
            qt, kt = built[b]
            state[b] = {"v": loads[b][2], "et": {}, "rr": {}}
            for i in range(NT):
                if pending:
                    emit_av_finish(pending.pop())
                for j in st_sched[b][i]:
                    emit_st(b, j, qt, kt)
                emit_s(b, i, qt, kt)
                if i == 8 and b + 1 < BPC:
                    loads[b + 1] = prep_loads(b + 1)
                # spread next batch's Q/K transposes over idle late iters so
                # the batch boundary starts clean
                if b + 1 < BPC and 11 <= i <= 14:
                    if b + 1 not in built:
                        built[b + 1] = (
                            p_qt.tile([128, S], BF16, tag="qt", name="qt_n"),
                            p_kt.tile([128, S], BF16, tag="kt", name="kt_n"),
                        )
                    qt1, kt1 = built[b + 1]
                    qb1, kb1, _ = loads[b + 1]
                    for g in (0, 1):
                        g2 = 2 * (i - 11) + g
                        src, dst = ((kb1, kt1), (qb1, qt1))[g2 // 4]
                        prep_group(src, dst, g2 % 4)
                if i in av_sched[b]:
                    pending.append(emit_av_start(*av_sched[b][i]))
            if b >= 1:
                del state[b - 1]
        while pending:
            emit_av_finish(pending.pop())
        emit_av_finish(emit_av_start(last, 14, 2))

    nc.compile()
    return nc


_NC_CACHE = None


def _get_nc():
    global _NC_CACHE
    if _NC_CACHE is None:
        _NC_CACHE = build_graph()
    return _NC_CACHE


def kernel(Q, K, V, _trace=False):
    Q = np.asarray(Q, dtype=np.float32)
    K = np.asarray(K, dtype=np.float32)
    V = np.asarray(V, dtype=np.float32)
    nc = _get_nc()
    in_maps = [
        {
            "Q": np.ascontiguousarray(Q[i * BPC : (i + 1) * BPC]),
            "K": np.ascontiguousarray(K[i * BPC : (i + 1) * BPC]),
            "V": np.ascontiguousarray(V[i * BPC : (i + 1) * BPC]),
        }
        for i in range(N_CORES)
    ]
    res = run_bass_kernel_spmd(nc, in_maps, core_ids=list(range(N_CORES)), trace=_trace)
    out = np.concatenate([res.results[i]["out"] for i in range(N_CORES)], axis=0)
    attn = np.concatenate([res.results[i]["attn"] for i in range(N_CORES)], axis=0)
    if _trace:
        kernel.last_exec_time_ns = res.exec_time_ns
    return out, attn


if __name__ == "__main__":
    rng = np.random.default_rng(0)
    Q = rng.standard_normal((B_FULL, S, D), dtype=np.float32)
    K = rng.standard_normal((B_FULL, S, D), dtype=np.float32)
    V = rng.standard_normal((B_FULL, S, D), dtype=np.float32)
    o, a = kernel(Q, K, V, _trace=True)
    print("out", o.shape, "attn", a.shape, "exec_ns", kernel.last_exec_time_ns)
